# revision 18
# baseline (speedup 1.0000x reference)
"""Multi-head attention (B=2, S=2048, H=1024, 16 heads) on 8 TRN2 NeuronCores.

Sharding: tensor-parallel over heads x data-parallel over batch.
core = b * 4 + g handles batch b and head-group g (4 heads, 256 channels).

Device-side dataflow (bf16 operands, fp32 PSUM accumulation):
  - Everything stays in "transposed space" so every matmul contracts over the
    partition dim with no on-device transposes:
      x      [H, S] swizzled to [P, tq, hc, 512]  (host pre-transposed)
      qk_T   [512, S]    = (Wqk_g x_t)            rows: q(4 heads), k(4 heads)
      v      [S, 256]    = x w_v.T  (natural layout; lhsT = x_t chunks)
      st     [128k, q]   = k_T_h^T-contracted scores (transposed scores)
      pt     = exp(st * scale + mask[k])          (ACT, bias = per-partition mask)
      av     [128, q]    = v_aug^T pt ; rows 0:64 = unnormalized out.T,
                           rows 64:128 = Z[q] replicated (v_aug cols 64:128 == 1)
      attn_T [256, S]    = av[:64] * reciprocal(av[64:128])
      out_t  [H, S]      = Wo_g^T-contracted partial output (transposed)
  - Host sums the 4 group partials per batch, transposes back, and adds the
    exact bias corrections: b_out plus w_out @ b_qkv[v-part].

Schedule: one flattened, software-pipelined stream over all 8 attention
windows (2 head-pairs x 4 q-windows of 512).  Slot t emits st(t) + exp(t) and
the AV pair for slot t-2, so the ACT engine (~1.05us/exp) never waits at
window boundaries.  QKV/V/out projections are emitted as fill work at fixed
slots so each tile is produced just ahead of its first consumer and the PE
never stalls on the exp latency.

All HBM tensors are host-swizzled to partition-major layouts so every DMA
moves one contiguous multi-KB run per partition (descriptor-efficient):
x streams in token-quarters on two DMA queues (first matmul starts ~5us in),
weights stream on the scalar queue in first-use order.
"""

import numpy as np

import concourse.tile as tile
from concourse import bacc, mybir
from concourse.bass_utils import run_bass_kernel_spmd

B, S, H = 2, 2048, 1024
NH, HD = 16, 64
NCORES = 8
NGROUP = 4              # head groups = cores per batch
HPG = NH // NGROUP      # 4 heads per group
DG = HPG * HD           # 256 channels per group
P = 128
SCALE = float(HD) ** -0.5

FP32 = mybir.dt.float32
BF16 = mybir.dt.bfloat16

S_TILES = S // P        # 16 key/token tiles
HC = H // P             # 8 contraction chunks over H
QKR = 2 * DG            # 512 q+k rows
QKC = QKR // P          # 4 chunks of qk rows
QT = 1024               # st tile width (two 512 q-windows, one per head)
NWIN = 8                # (head-pair, q-window) attention windows
NT = NWIN * S_TILES     # 128 pipeline slots
LAG = 2                 # av trails st by this many slots

_NC_CACHE = None
LAST_RESULT = None      # BassKernelResults of the most recent run (for test.py)


def _body(tc, x_t, wqk_t, wv_t, wo_t, bqk, mask, out_t):
    nc = tc.nc
    with (
        tc.tile_pool(name="const", bufs=1) as const,
        tc.tile_pool(name="big", bufs=1) as big,
        tc.tile_pool(name="pt_pool", bufs=8) as pt_pool,
        tc.tile_pool(name="rz_pool", bufs=2) as rz_pool,
        tc.tile_pool(name="osb_pool", bufs=2) as osb_pool,
        tc.tile_pool(name="ps", bufs=2, space="PSUM") as ps,
        tc.tile_pool(name="avps", bufs=2, space="PSUM") as avps,
        tc.tile_pool(name="iops", bufs=2, space="PSUM") as iops,
    ):
        # ---------- input DMAs, in first-use order ----------
        # Every HBM tensor is partition-major and per-partition contiguous,
        # so each dma_start is one big descriptor per partition.
        # quarter 0 lands in two hc-halves so the init projections can start
        # contracting hc 0-3 while hc 4-7 is still in flight.
        x_sb = big.tile([P, 4, HC, 512], BF16, name="x_sb")
        nc.sync.dma_start(x_sb[:, 0, 0:4], x_t[:, 0, 0:4])
        nc.sync.dma_start(x_sb[:, 0, 4:8], x_t[:, 0, 4:8])
        nc.gpsimd.dma_start(x_sb[:, 1], x_t[:, 1])
        nc.sync.dma_start(x_sb[:, 2], x_t[:, 2])
        nc.gpsimd.dma_start(x_sb[:, 3], x_t[:, 3])

        wqk_sb = const.tile([P, QKC, HC, P], BF16, name="wqk_sb")
        nc.scalar.dma_start(wqk_sb[:, 0], wqk_t[:, 0])    # q pair0
        nc.scalar.dma_start(wqk_sb[:, 2], wqk_t[:, 2])    # k pair0
        wv_sb = const.tile([P, HC, DG], BF16, name="wv_sb")
        nc.scalar.dma_start(wv_sb[:], wv_t[:])
        bqk_sb = const.tile([P, QKC], FP32, name="bqk_sb")
        nc.scalar.dma_start(bqk_sb[:], bqk[:])
        mask_sb = const.tile([P, S_TILES], FP32, name="mask_sb")
        nc.scalar.dma_start(mask_sb[:], mask[:])
        nc.scalar.dma_start(wqk_sb[:, 1], wqk_t[:, 1])    # q pair1
        nc.scalar.dma_start(wqk_sb[:, 3], wqk_t[:, 3])    # k pair1
        wo_sb = const.tile([P, DG // P, H], BF16, name="wo_sb")
        nc.scalar.dma_start(wo_sb[:], wo_t[:])

        qk_sb = big.tile([P, QKC, S], BF16, name="qk_sb")
        # v_aug: per token tile / head: [v (64 cols) | ones (64 cols)]
        v_sb = big.tile([P, S_TILES, HPG, 2 * HD], BF16, name="v_sb")
        attn_sb = big.tile([P, DG // P, S], BF16, name="attn_sb")

        # ones half of v_aug: memset a bf16 staging tile, copy per token tile
        ones_sb = const.tile([P, HPG, HD], BF16, name="ones_sb")
        nc.vector.memset(ones_sb[:], 1.0)
        for tt in range(S_TILES):
            nc.vector.tensor_copy(v_sb[:, tt, :, HD:2 * HD], ones_sb[:])

        # ---------- projection / output fill units ----------
        def vg(tp):
            # two token tiles (2*tp, 2*tp+1) side by side in one psum slot
            v_ps = iops.tile([P, 512], FP32, name="v_ps", tag="io")
            for half in range(2):
                tq, off = divmod(256 * tp + 128 * half, 512)
                for hc in range(HC):
                    nc.tensor.matmul(
                        v_ps[:, half * DG:(half + 1) * DG],
                        lhsT=x_sb[:, tq, hc, off:off + P],
                        rhs=wv_sb[:, hc, :],
                        start=(hc == 0),
                        stop=(hc == HC - 1),
                    )
            nc.vector.tensor_copy(
                v_sb[:, 2 * tp:2 * tp + 2, :, 0:HD],
                v_ps[:].rearrange("p (t h d) -> p t h d", t=2, d=HD),
            )

        def qkg_part(rc, i, state, lo, hi):
            if lo == 0:
                state[0] = iops.tile([P, 512], FP32, name="qk_ps", tag="io")
            qk_ps = state[0]
            for hc in range(lo, hi):
                nc.tensor.matmul(
                    qk_ps[:],
                    lhsT=wqk_sb[:, rc, hc, :],
                    rhs=x_sb[:, i, hc, :],
                    start=(hc == 0),
                    stop=(hc == HC - 1),
                )
            if hi == HC:
                nc.vector.tensor_scalar_add(
                    qk_sb[:, rc, i * 512:(i + 1) * 512],
                    qk_ps[:],
                    bqk_sb[:, rc:rc + 1],
                )

        def qkg(rc, i):
            qkg_part(rc, i, [None], 0, HC)

        def qkg2(rc, i):
            """qkg split into two fill units of 4 matmuls each."""
            state = [None]
            return (lambda: qkg_part(rc, i, state, 0, HC // 2),
                    lambda: qkg_part(rc, i, state, HC // 2, HC))

        def dummy():
            # pure PE filler: keeps the PE pipeline/activity-monitor hot in
            # fill-starved stretches so real matmuls don't pay the
            # post-stall pipeline-refill penalty.
            warm = iops.tile([P, 512], FP32, name="warm", tag="io")
            nc.tensor.matmul(warm[:], lhsT=wv_sb[:, 0, 0:P],
                             rhs=x_sb[:, 0, 0, :], start=True, stop=True)

        o_stage = [None]

        def out_piece(q5, j, tail=False):
            qlo = q5 * 512
            if j == 0:
                o_stage[0] = osb_pool.tile([P, HC, 512], BF16,
                                           name="o_stage", tag="osb")
            o_ps = iops.tile([P, 512], FP32, name="o_ps", tag="io")
            for kc in range(DG // P):
                nc.tensor.matmul(
                    o_ps[:],
                    lhsT=wo_sb[:, kc, j * P:(j + 1) * P],
                    rhs=attn_sb[:, kc, qlo:qlo + 512],
                    start=(kc == 0),
                    stop=(kc == DG // P - 1),
                )
            if tail and j % 2 == 0:
                # final window: split the stage copies between the (idle)
                # scalar engine and DVE so the copy chain is not serial on
                # one engine behind the last matmuls.
                nc.scalar.copy(o_stage[0][:, j, :], o_ps[:])
            else:
                nc.vector.tensor_copy(o_stage[0][:, j, :], o_ps[:])
            if j == 3:
                nc.sync.dma_start(out_t[:, q5, 0:4], o_stage[0][:, 0:4, :])
            elif j == HC - 1:
                nc.sync.dma_start(out_t[:, q5, 4:8], o_stage[0][:, 4:8, :])

        # fill slots: each unit lands just ahead of its first consumer, and
        # every ACT-paced window carries ~4.5us of fill so the PE never
        # drains between the exp-gated AV matmuls.  Split-qkg halves stay
        # adjacent (their open PSUM accumulator shares the 2-deep io pool
        # with the other fill units).
        fill_at = {}

        def add(t, *fs):
            fill_at.setdefault(t, []).extend(fs)

        # window (0,0): k/v production for all 16 token tiles
        add(0, lambda: qkg(2, 1))
        add(1, lambda: vg(2))
        add(2, lambda: vg(3))
        add(3, lambda: qkg(2, 2))
        add(4, lambda: vg(4))
        add(5, lambda: vg(5))
        add(6, lambda: qkg(2, 3))
        add(7, lambda: vg(6))
        add(8, lambda: vg(7))
        for (rc, i), s in {
            (0, 1): 12,   # q win1, needed t=16
            (3, 0): 18,   # k pair1, needed t=64
            (0, 2): 24,   # needed t=32
            (3, 1): 34,   # needed t=68
            (0, 3): 40,   # needed t=48
            (1, 0): 50,   # q pair1 win0, needed t=64
            (3, 2): 54,   # needed t=72
            (3, 3): 68,   # needed t=76 (fills window (1,0))
            (1, 1): 62,   # needed t=80
            (1, 2): 76,   # needed t=96 (fills window (1,0))
            (1, 3): 97,   # needed t=112
        }.items():
            add(s, lambda rc=rc, i=i: qkg(rc, i))
        for j in range(8):
            add(83 + j, lambda j=j: out_piece(0, j))
            add(99 + j, lambda j=j: out_piece(1, j))
            add(min(114 + 2 * j, 127), lambda j=j: out_piece(2, j))

        # ---------- attention pipeline ----------
        # Heads (2*qc, 2*qc+1) live at partition offsets 0/64 of qk chunk qc,
        # so their score matmuls land in disjoint PE row groups and
        # co-execute.  Their 512-wide score tiles sit side by side in one
        # [128,1024] PSUM tile so a single N=1024 exp covers both.
        pts = {}
        avs = {}

        def emit_st(t):
            w, kt = divmod(t, S_TILES)
            qc, q5 = divmod(w, 4)
            qlo = q5 * 512
            st = ps.tile([P, QT], FP32, name="st", tag="mm")
            for half in range(2):
                off = half * HD
                nc.tensor.matmul(
                    st[:, half * 512:(half + 1) * 512],
                    lhsT=qk_sb[off:off + HD, 2 + qc, kt * P:(kt + 1) * P],
                    rhs=qk_sb[off:off + HD, qc, qlo:qlo + 512],
                    start=True,
                    stop=True,
                )
            pt = pt_pool.tile([P, QT], BF16, name="pt", tag="pt")
            nc.scalar.activation(
                pt[:], st[:],
                mybir.ActivationFunctionType.Exp,
                bias=mask_sb[:, kt:kt + 1],
                scale=SCALE,
            )
            pts[t] = pt

        def emit_av(t):
            w, kt = divmod(t, S_TILES)
            qc, q5 = divmod(w, 4)
            if kt == 0:
                avs[w] = (avps.tile([P, 512], FP32, name="av0", tag="av"),
                          avps.tile([P, 512], FP32, name="av1", tag="av"))
            pt = pts.pop(t)
            for half, av in ((0, avs[w][0]), (1, avs[w][1])):
                nc.tensor.matmul(
                    av[:],
                    lhsT=v_sb[:, kt, 2 * qc + half, :],
                    rhs=pt[:, half * 512:(half + 1) * 512],
                    start=(kt == 0),
                    stop=(kt == S_TILES - 1),
                )
            if kt == S_TILES - 1:
                qlo = q5 * 512
                for half, av in ((0, avs[w][0]), (1, avs[w][1])):
                    off = half * HD
                    zc = rz_pool.tile([HD, 512], FP32, name="zc", tag="zc")
                    # Z staging copy on the scalar engine: it fits in ACT's
                    # window-boundary gap and keeps DVE free for the
                    # out-projection stage copies (which gate PE psum slots).
                    nc.scalar.copy(zc[:], av[HD:2 * HD, :])
                    rz = rz_pool.tile([HD, 512], FP32, name="rz", tag="rz")
                    nc.vector.reciprocal_approx_fast(rz[:], zc[:])
                    nc.vector.tensor_mul(
                        attn_sb[off:off + HD, qc, qlo:qlo + 512],
                        av[0:HD, :],
                        rz[:],
                    )
                del avs[w]

        # init: everything window (0,0) kt 0-3 needs.  The two qk groups run
        # as hc-halves so hc 0-3 matmuls overlap the hc 4-7 x-quarter DMA.
        s_q, s_k = [None], [None]
        qkg_part(0, 0, s_q, 0, HC // 2)
        qkg_part(2, 0, s_k, 0, HC // 2)
        qkg_part(0, 0, s_q, HC // 2, HC)
        qkg_part(2, 0, s_k, HC // 2, HC)
        vg(0)
        vg(1)

        for t in range(NT + LAG):
            if t < NT:
                emit_st(t)
            for f in fill_at.get(t, ()):
                f()
            if t >= LAG:
                emit_av(t - LAG)

        # keep the PE activity monitor hot while the final window's
        # normalization runs on DVE, so the tail out-projection executes at
        # full clock instead of the cold 1.2 GHz p-state.
        for _ in range(6):
            dummy()
        for j in range(HC):
            out_piece(3, j, tail=True)


def _build():
    nc = bacc.Bacc(
        "TRN2",
        target_bir_lowering=False,
        debug=False,
        enable_asserts=True,
        num_devices=NCORES,
    )
    x_t = nc.dram_tensor("x_t", [P, 4, HC, 512], BF16, kind="ExternalInput").ap()
    wqk_t = nc.dram_tensor("wqk_t", [P, QKC, HC, P], BF16,
                           kind="ExternalInput").ap()
    wv_t = nc.dram_tensor("wv_t", [P, HC, DG], BF16, kind="ExternalInput").ap()
    wo_t = nc.dram_tensor("wo_t", [P, DG // P, H], BF16,
                          kind="ExternalInput").ap()
    bqk = nc.dram_tensor("bqk", [P, QKC], FP32, kind="ExternalInput").ap()
    mask = nc.dram_tensor("mask", [P, S_TILES], FP32, kind="ExternalInput").ap()
    out_t = nc.dram_tensor("out_t", [P, 4, HC, 512], BF16,
                           kind="ExternalOutput").ap()

    with tile.TileContext(nc) as tc:
        _body(tc, x_t, wqk_t, wv_t, wo_t, bqk, mask, out_t)
    nc.compile()
    return nc


def _get_nc():
    global _NC_CACHE
    if _NC_CACHE is None:
        _NC_CACHE = _build()
    return _NC_CACHE


def _swz(a, inner):
    """[H?, W] -> partition-major [(P), blocks, W-chunks...] host swizzle:
    reshape rows (c p) -> [c, P, ...] then move P first."""
    c = a.shape[0] // P
    r = a.reshape(c, P, *inner)
    order = (1, 0) + tuple(range(2, r.ndim))
    return np.ascontiguousarray(r.transpose(order))


def make_in_maps(hidden_states, attention_mask, w_qkv, b_qkv, w_out):
    import ml_dtypes

    bf16 = ml_dtypes.bfloat16
    in_maps = []
    for core in range(NCORES):
        b, g = divmod(core, NGROUP)
        wq = w_qkv[0 * H + g * DG:0 * H + (g + 1) * DG]
        wk = w_qkv[1 * H + g * DG:1 * H + (g + 1) * DG]
        wv = w_qkv[2 * H + g * DG:2 * H + (g + 1) * DG]

        # x: [H, S] -> [P, tq, hc, 512]
        xt = np.asarray(hidden_states[b].T)                     # [H, S]
        x4 = xt.reshape(HC, P, 4, 512).transpose(1, 2, 0, 3)
        # wqk: [H, 512] -> [P, rc, hc, 128]
        wqk_t = np.concatenate([wq, wk], 0).T                   # [H, 512]
        wqk4 = wqk_t.reshape(HC, P, QKC, P).transpose(1, 2, 0, 3)
        # wv: [H, 256] -> [P, hc, 256]
        wv3 = wv.T.reshape(HC, P, DG).transpose(1, 0, 2)
        # wo: [256, H] -> [P, kc, H]
        wo3 = w_out[:, g * DG:(g + 1) * DG].T.reshape(2, P, H).transpose(1, 0, 2)

        bqk_cat = np.concatenate([b_qkv[g * DG:(g + 1) * DG],
                                  b_qkv[H + g * DG:H + (g + 1) * DG]])
        in_maps.append({
            "x_t": np.ascontiguousarray(x4).astype(bf16),
            "wqk_t": np.ascontiguousarray(wqk4).astype(bf16),
            "wv_t": np.ascontiguousarray(wv3).astype(bf16),
            "wo_t": np.ascontiguousarray(wo3).astype(bf16),
            "bqk": np.ascontiguousarray(bqk_cat.reshape(QKC, P).T),
            "mask": np.ascontiguousarray(
                np.asarray(attention_mask[b]).reshape(S_TILES, P).T),
        })
    return in_maps


def unswizzle_out(arr):
    """[P, q5, hc, 512] -> [H, S]"""
    return np.asarray(arr).transpose(2, 0, 1, 3).reshape(H, S)


def kernel(hidden_states, attention_mask, w_qkv, b_qkv, w_out, b_out):
    global LAST_RESULT
    hidden_states = np.asarray(hidden_states, dtype=np.float32)
    attention_mask = np.asarray(attention_mask, dtype=np.float32)
    w_qkv = np.asarray(w_qkv, dtype=np.float32)
    b_qkv = np.asarray(b_qkv, dtype=np.float32)
    w_out = np.asarray(w_out, dtype=np.float32)
    b_out = np.asarray(b_out, dtype=np.float32)

    nc = _get_nc()
    in_maps = make_in_maps(hidden_states, attention_mask, w_qkv, b_qkv, w_out)

    import os
    trace = bool(int(os.environ.get("KERNEL_TRACE", "0")))
    res = run_bass_kernel_spmd(
        nc, in_maps, core_ids=list(range(NCORES)), trace=trace,
    )
    LAST_RESULT = res

    out = np.zeros((B, S, H), np.float32)
    vbias = w_out @ b_qkv[2 * H:]          # exact v-bias correction
    for b in range(B):
        acc = unswizzle_out(res.results[b * NGROUP + 0]["out_t"]).astype(
            np.float32)
        for g in range(1, NGROUP):
            acc = acc + unswizzle_out(
                res.results[b * NGROUP + g]["out_t"]).astype(np.float32)
        out[b] = acc.T + b_out + vbias
    return out


# revision 20
# speedup vs baseline: 1.0607x; 1.0607x over previous
"""Multi-head attention (B=2, S=2048, H=1024, 16 heads) on 8 TRN2 NeuronCores.

Sharding: tensor-parallel over heads x data-parallel over batch.
core = b * 4 + g handles batch b and head-group g (4 heads, 256 channels).

Device-side dataflow (bf16 operands, fp32 PSUM accumulation):
  - Everything stays in "transposed space" so every matmul contracts over the
    partition dim with no on-device transposes:
      x      [H, S] swizzled to [P, tq, hc, 512]  (host pre-transposed)
      qk_T   [512, S]    = (Wqk_g x_t)            rows: q(4 heads), k(4 heads)
      v      [S, 256]    = x w_v.T  (natural layout; lhsT = x_t chunks)
      st     [128k, q]   = k_T_h^T-contracted scores (transposed scores)
      pt     = exp(st * scale + mask[k])          (ACT, bias = per-partition mask)
      av     [128, q]    = v_aug^T pt ; rows 0:64 = unnormalized out.T,
                           rows 64:128 = Z[q] replicated (v_aug cols 64:128 == 1)
      attn_T [256, S]    = av[:64] * reciprocal(av[64:128])
      out_t  [H, S]      = Wo_g^T-contracted partial output (transposed)
  - Host sums the 4 group partials per batch, transposes back, and adds the
    exact bias corrections: b_out plus w_out @ b_qkv[v-part].

Schedule: one flattened, software-pipelined stream over all 8 attention
windows (2 head-pairs x 4 q-windows of 512).  Slot t emits st(t) + exp(t) and
the AV pair for slot t-2, so the ACT engine (~1.05us/exp) never waits at
window boundaries.  QKV/V/out projections are emitted as fill work at fixed
slots so each tile is produced just ahead of its first consumer and the PE
never stalls on the exp latency.

All HBM tensors are host-swizzled to partition-major layouts so every DMA
moves one contiguous multi-KB run per partition (descriptor-efficient):
x streams in token-quarters on two DMA queues (first matmul starts ~5us in),
weights stream on the scalar queue in first-use order.
"""

import numpy as np

import concourse.tile as tile
from concourse import bacc, mybir
from concourse.bass_utils import run_bass_kernel_spmd

B, S, H = 2, 2048, 1024
NH, HD = 16, 64
NCORES = 8
NGROUP = 4              # head groups = cores per batch
HPG = NH // NGROUP      # 4 heads per group
DG = HPG * HD           # 256 channels per group
P = 128
SCALE = float(HD) ** -0.5

FP32 = mybir.dt.float32
BF16 = mybir.dt.bfloat16

S_TILES = S // P        # 16 key/token tiles
HC = H // P             # 8 contraction chunks over H
QKR = 2 * DG            # 512 q+k rows
QKC = QKR // P          # 4 chunks of qk rows
QT = 1024               # st tile width (two 512 q-windows, one per head)
NWIN = 8                # (head-pair, q-window) attention windows
NT = NWIN * S_TILES     # 128 pipeline slots
LAG = 2                 # av trails st by this many slots

_NC_CACHE = None
LAST_RESULT = None      # BassKernelResults of the most recent run (for test.py)


def _body(tc, x_t, wqk_t, wv_t, wo_t, bqk, mask, out_t):
    nc = tc.nc
    with (
        tc.tile_pool(name="const", bufs=1) as const,
        tc.tile_pool(name="big", bufs=1) as big,
        tc.tile_pool(name="pt_pool", bufs=8) as pt_pool,
        tc.tile_pool(name="rz_pool", bufs=2) as rz_pool,
        tc.tile_pool(name="osb_pool", bufs=2) as osb_pool,
        tc.tile_pool(name="ps", bufs=2, space="PSUM") as ps,
        tc.tile_pool(name="avps", bufs=2, space="PSUM") as avps,
        tc.tile_pool(name="iops", bufs=2, space="PSUM") as iops,
    ):
        # ---------- input DMAs, in first-use order ----------
        # Every HBM tensor is partition-major and per-partition contiguous,
        # so each dma_start is one big descriptor per partition.
        # quarter 0 lands in two hc-halves so the init projections can start
        # contracting hc 0-3 while hc 4-7 is still in flight.
        # all x on the sync ring, serially: quarter 0 only contends with the
        # (small) first weight DMAs, so the first matmul starts ~5us in;
        # later quarters still land well ahead of their first consumers
        # (quarter q at ~3q us vs first use at ~+16us into the stream).
        x_sb = big.tile([P, 4, HC, 512], BF16, name="x_sb")
        nc.sync.dma_start(x_sb[:, 0, 0:4], x_t[:, 0, 0:4])
        nc.sync.dma_start(x_sb[:, 0, 4:8], x_t[:, 0, 4:8])
        for tq in range(1, 4):
            nc.sync.dma_start(x_sb[:, tq], x_t[:, tq])

        wqk_sb = const.tile([P, QKC, HC, P], BF16, name="wqk_sb")
        nc.scalar.dma_start(wqk_sb[:, 0], wqk_t[:, 0])    # q pair0
        nc.scalar.dma_start(wqk_sb[:, 2], wqk_t[:, 2])    # k pair0
        wv_sb = const.tile([P, HC, DG], BF16, name="wv_sb")
        nc.scalar.dma_start(wv_sb[:], wv_t[:])
        bqk_sb = const.tile([P, QKC], FP32, name="bqk_sb")
        nc.scalar.dma_start(bqk_sb[:], bqk[:])
        mask_sb = const.tile([P, S_TILES], FP32, name="mask_sb")
        nc.scalar.dma_start(mask_sb[:], mask[:])
        nc.scalar.dma_start(wqk_sb[:, 1], wqk_t[:, 1])    # q pair1
        nc.scalar.dma_start(wqk_sb[:, 3], wqk_t[:, 3])    # k pair1
        wo_sb = const.tile([P, DG // P, H], BF16, name="wo_sb")
        nc.scalar.dma_start(wo_sb[:], wo_t[:])

        qk_sb = big.tile([P, QKC, S], BF16, name="qk_sb")
        # v_aug: per token tile / head: [v (64 cols) | ones (64 cols)]
        v_sb = big.tile([P, S_TILES, HPG, 2 * HD], BF16, name="v_sb")
        attn_sb = big.tile([P, DG // P, S], BF16, name="attn_sb")

        # ones half of v_aug: memset a bf16 staging tile, copy per token tile
        ones_sb = const.tile([P, HPG, HD], BF16, name="ones_sb")
        nc.vector.memset(ones_sb[:], 1.0)
        for tt in range(S_TILES):
            nc.vector.tensor_copy(v_sb[:, tt, :, HD:2 * HD], ones_sb[:])

        # ---------- projection / output fill units ----------
        def vg(tp):
            # two token tiles (2*tp, 2*tp+1) side by side in one psum slot
            v_ps = iops.tile([P, 512], FP32, name="v_ps", tag="io")
            for half in range(2):
                tq, off = divmod(256 * tp + 128 * half, 512)
                for hc in range(HC):
                    nc.tensor.matmul(
                        v_ps[:, half * DG:(half + 1) * DG],
                        lhsT=x_sb[:, tq, hc, off:off + P],
                        rhs=wv_sb[:, hc, :],
                        start=(hc == 0),
                        stop=(hc == HC - 1),
                    )
            nc.vector.tensor_copy(
                v_sb[:, 2 * tp:2 * tp + 2, :, 0:HD],
                v_ps[:].rearrange("p (t h d) -> p t h d", t=2, d=HD),
            )

        def qkg_part(rc, i, state, lo, hi):
            if lo == 0:
                state[0] = iops.tile([P, 512], FP32, name="qk_ps", tag="io")
            qk_ps = state[0]
            for hc in range(lo, hi):
                nc.tensor.matmul(
                    qk_ps[:],
                    lhsT=wqk_sb[:, rc, hc, :],
                    rhs=x_sb[:, i, hc, :],
                    start=(hc == 0),
                    stop=(hc == HC - 1),
                )
            if hi == HC:
                nc.vector.tensor_scalar_add(
                    qk_sb[:, rc, i * 512:(i + 1) * 512],
                    qk_ps[:],
                    bqk_sb[:, rc:rc + 1],
                )

        def qkg(rc, i):
            qkg_part(rc, i, [None], 0, HC)

        def qkg2(rc, i):
            """qkg split into two fill units of 4 matmuls each."""
            state = [None]
            return (lambda: qkg_part(rc, i, state, 0, HC // 2),
                    lambda: qkg_part(rc, i, state, HC // 2, HC))

        def dummy():
            # pure PE filler: keeps the PE pipeline/activity-monitor hot in
            # fill-starved stretches so real matmuls don't pay the
            # post-stall pipeline-refill penalty.
            warm = iops.tile([P, 512], FP32, name="warm", tag="io")
            nc.tensor.matmul(warm[:], lhsT=wv_sb[:, 0, 0:P],
                             rhs=x_sb[:, 0, 0, :], start=True, stop=True)

        o_stage = [None]

        def out_piece(q5, j, tail=False):
            qlo = q5 * 512
            if j == 0:
                o_stage[0] = osb_pool.tile([P, HC, 512], BF16,
                                           name="o_stage", tag="osb")
            o_ps = iops.tile([P, 512], FP32, name="o_ps", tag="io")
            for kc in range(DG // P):
                nc.tensor.matmul(
                    o_ps[:],
                    lhsT=wo_sb[:, kc, j * P:(j + 1) * P],
                    rhs=attn_sb[:, kc, qlo:qlo + 512],
                    start=(kc == 0),
                    stop=(kc == DG // P - 1),
                )
            if tail and j % 2 == 0:
                # final window: split the stage copies between the (idle)
                # scalar engine and DVE so the copy chain is not serial on
                # one engine behind the last matmuls.
                nc.scalar.copy(o_stage[0][:, j, :], o_ps[:])
            else:
                nc.vector.tensor_copy(o_stage[0][:, j, :], o_ps[:])
            if j == 3:
                nc.sync.dma_start(out_t[:, q5, 0:4], o_stage[0][:, 0:4, :])
            elif j == HC - 1:
                nc.sync.dma_start(out_t[:, q5, 4:8], o_stage[0][:, 4:8, :])

        # fill slots: each unit lands just ahead of its first consumer, and
        # every ACT-paced window carries ~4.5us of fill so the PE never
        # drains between the exp-gated AV matmuls.  Split-qkg halves stay
        # adjacent (their open PSUM accumulator shares the 2-deep io pool
        # with the other fill units).
        fill_at = {}

        def add(t, *fs):
            fill_at.setdefault(t, []).extend(fs)

        # window (0,0): k/v production for all 16 token tiles
        add(0, lambda: qkg(2, 1))
        add(1, lambda: vg(2))
        add(2, lambda: vg(3))
        add(3, lambda: qkg(2, 2))
        add(4, lambda: vg(4))
        add(5, lambda: vg(5))
        add(6, lambda: qkg(2, 3))
        add(7, lambda: vg(6))
        add(8, lambda: vg(7))
        for (rc, i), s in {
            (0, 1): 12,   # q win1, needed t=16
            (3, 0): 18,   # k pair1, needed t=64
            (0, 2): 24,   # needed t=32
            (3, 1): 34,   # needed t=68
            (0, 3): 40,   # needed t=48
            (1, 0): 50,   # q pair1 win0, needed t=64
            (3, 2): 54,   # needed t=72
            (3, 3): 68,   # needed t=76 (fills window (1,0))
            (1, 1): 62,   # needed t=80
            (1, 2): 76,   # needed t=96 (fills window (1,0))
            (1, 3): 97,   # needed t=112
        }.items():
            add(s, lambda rc=rc, i=i: qkg(rc, i))
        for j in range(8):
            add(83 + j, lambda j=j: out_piece(0, j))
            add(99 + j, lambda j=j: out_piece(1, j))
            add(min(114 + 2 * j, 127), lambda j=j: out_piece(2, j))

        # ---------- attention pipeline ----------
        # Heads (2*qc, 2*qc+1) live at partition offsets 0/64 of qk chunk qc,
        # so their score matmuls land in disjoint PE row groups and
        # co-execute.  Their 512-wide score tiles sit side by side in one
        # [128,1024] PSUM tile so a single N=1024 exp covers both.
        pts = {}
        avs = {}

        def emit_st(t):
            w, kt = divmod(t, S_TILES)
            qc, q5 = divmod(w, 4)
            qlo = q5 * 512
            st = ps.tile([P, QT], FP32, name="st", tag="mm")
            for half in range(2):
                off = half * HD
                nc.tensor.matmul(
                    st[:, half * 512:(half + 1) * 512],
                    lhsT=qk_sb[off:off + HD, 2 + qc, kt * P:(kt + 1) * P],
                    rhs=qk_sb[off:off + HD, qc, qlo:qlo + 512],
                    start=True,
                    stop=True,
                )
            pt = pt_pool.tile([P, QT], BF16, name="pt", tag="pt")
            nc.scalar.activation(
                pt[:], st[:],
                mybir.ActivationFunctionType.Exp,
                bias=mask_sb[:, kt:kt + 1],
                scale=SCALE,
            )
            pts[t] = pt

        def emit_av(t):
            w, kt = divmod(t, S_TILES)
            qc, q5 = divmod(w, 4)
            if kt == 0:
                avs[w] = (avps.tile([P, 512], FP32, name="av0", tag="av"),
                          avps.tile([P, 512], FP32, name="av1", tag="av"))
            pt = pts.pop(t)
            for half, av in ((0, avs[w][0]), (1, avs[w][1])):
                nc.tensor.matmul(
                    av[:],
                    lhsT=v_sb[:, kt, 2 * qc + half, :],
                    rhs=pt[:, half * 512:(half + 1) * 512],
                    start=(kt == 0),
                    stop=(kt == S_TILES - 1),
                )
            if kt == S_TILES - 1:
                qlo = q5 * 512
                for half, av in ((0, avs[w][0]), (1, avs[w][1])):
                    off = half * HD
                    zc = rz_pool.tile([HD, 512], FP32, name="zc", tag="zc")
                    nc.vector.tensor_copy(zc[:], av[HD:2 * HD, :])
                    rz = rz_pool.tile([HD, 512], FP32, name="rz", tag="rz")
                    nc.vector.reciprocal_approx_fast(rz[:], zc[:])
                    nc.vector.tensor_mul(
                        attn_sb[off:off + HD, qc, qlo:qlo + 512],
                        av[0:HD, :],
                        rz[:],
                    )
                del avs[w]

        # init: everything window (0,0) kt 0-3 needs.  The two qk groups run
        # as hc-halves so hc 0-3 matmuls overlap the hc 4-7 x-quarter DMA.
        s_q, s_k = [None], [None]
        qkg_part(0, 0, s_q, 0, HC // 2)
        qkg_part(2, 0, s_k, 0, HC // 2)
        qkg_part(0, 0, s_q, HC // 2, HC)
        qkg_part(2, 0, s_k, HC // 2, HC)
        vg(0)
        vg(1)

        for t in range(NT + LAG):
            if t < NT:
                emit_st(t)
            for f in fill_at.get(t, ()):
                f()
            if t >= LAG:
                emit_av(t - LAG)

        # keep the PE activity monitor hot while the final window's
        # normalization runs on DVE, so the tail out-projection executes at
        # full clock instead of the cold 1.2 GHz p-state.
        for _ in range(6):
            dummy()
        for j in range(HC):
            out_piece(3, j, tail=True)


def _build():
    nc = bacc.Bacc(
        "TRN2",
        target_bir_lowering=False,
        debug=False,
        enable_asserts=True,
        num_devices=NCORES,
    )
    x_t = nc.dram_tensor("x_t", [P, 4, HC, 512], BF16, kind="ExternalInput").ap()
    wqk_t = nc.dram_tensor("wqk_t", [P, QKC, HC, P], BF16,
                           kind="ExternalInput").ap()
    wv_t = nc.dram_tensor("wv_t", [P, HC, DG], BF16, kind="ExternalInput").ap()
    wo_t = nc.dram_tensor("wo_t", [P, DG // P, H], BF16,
                          kind="ExternalInput").ap()
    bqk = nc.dram_tensor("bqk", [P, QKC], FP32, kind="ExternalInput").ap()
    mask = nc.dram_tensor("mask", [P, S_TILES], FP32, kind="ExternalInput").ap()
    out_t = nc.dram_tensor("out_t", [P, 4, HC, 512], BF16,
                           kind="ExternalOutput").ap()

    with tile.TileContext(nc) as tc:
        _body(tc, x_t, wqk_t, wv_t, wo_t, bqk, mask, out_t)
    nc.compile()
    return nc


def _get_nc():
    global _NC_CACHE
    if _NC_CACHE is None:
        _NC_CACHE = _build()
    return _NC_CACHE


def _swz(a, inner):
    """[H?, W] -> partition-major [(P), blocks, W-chunks...] host swizzle:
    reshape rows (c p) -> [c, P, ...] then move P first."""
    c = a.shape[0] // P
    r = a.reshape(c, P, *inner)
    order = (1, 0) + tuple(range(2, r.ndim))
    return np.ascontiguousarray(r.transpose(order))


def make_in_maps(hidden_states, attention_mask, w_qkv, b_qkv, w_out):
    import ml_dtypes

    bf16 = ml_dtypes.bfloat16
    in_maps = []
    for core in range(NCORES):
        b, g = divmod(core, NGROUP)
        wq = w_qkv[0 * H + g * DG:0 * H + (g + 1) * DG]
        wk = w_qkv[1 * H + g * DG:1 * H + (g + 1) * DG]
        wv = w_qkv[2 * H + g * DG:2 * H + (g + 1) * DG]

        # x: [H, S] -> [P, tq, hc, 512]
        xt = np.asarray(hidden_states[b].T)                     # [H, S]
        x4 = xt.reshape(HC, P, 4, 512).transpose(1, 2, 0, 3)
        # wqk: [H, 512] -> [P, rc, hc, 128]
        wqk_t = np.concatenate([wq, wk], 0).T                   # [H, 512]
        wqk4 = wqk_t.reshape(HC, P, QKC, P).transpose(1, 2, 0, 3)
        # wv: [H, 256] -> [P, hc, 256]
        wv3 = wv.T.reshape(HC, P, DG).transpose(1, 0, 2)
        # wo: [256, H] -> [P, kc, H]
        wo3 = w_out[:, g * DG:(g + 1) * DG].T.reshape(2, P, H).transpose(1, 0, 2)

        bqk_cat = np.concatenate([b_qkv[g * DG:(g + 1) * DG],
                                  b_qkv[H + g * DG:H + (g + 1) * DG]])
        in_maps.append({
            "x_t": np.ascontiguousarray(x4).astype(bf16),
            "wqk_t": np.ascontiguousarray(wqk4).astype(bf16),
            "wv_t": np.ascontiguousarray(wv3).astype(bf16),
            "wo_t": np.ascontiguousarray(wo3).astype(bf16),
            "bqk": np.ascontiguousarray(bqk_cat.reshape(QKC, P).T),
            "mask": np.ascontiguousarray(
                np.asarray(attention_mask[b]).reshape(S_TILES, P).T),
        })
    return in_maps


def unswizzle_out(arr):
    """[P, q5, hc, 512] -> [H, S]"""
    return np.asarray(arr).transpose(2, 0, 1, 3).reshape(H, S)


def kernel(hidden_states, attention_mask, w_qkv, b_qkv, w_out, b_out):
    global LAST_RESULT
    hidden_states = np.asarray(hidden_states, dtype=np.float32)
    attention_mask = np.asarray(attention_mask, dtype=np.float32)
    w_qkv = np.asarray(w_qkv, dtype=np.float32)
    b_qkv = np.asarray(b_qkv, dtype=np.float32)
    w_out = np.asarray(w_out, dtype=np.float32)
    b_out = np.asarray(b_out, dtype=np.float32)

    nc = _get_nc()
    in_maps = make_in_maps(hidden_states, attention_mask, w_qkv, b_qkv, w_out)

    import os
    trace = bool(int(os.environ.get("KERNEL_TRACE", "0")))
    res = run_bass_kernel_spmd(
        nc, in_maps, core_ids=list(range(NCORES)), trace=trace,
    )
    LAST_RESULT = res

    out = np.zeros((B, S, H), np.float32)
    vbias = w_out @ b_qkv[2 * H:]          # exact v-bias correction
    for b in range(B):
        acc = unswizzle_out(res.results[b * NGROUP + 0]["out_t"]).astype(
            np.float32)
        for g in range(1, NGROUP):
            acc = acc + unswizzle_out(
                res.results[b * NGROUP + g]["out_t"]).astype(np.float32)
        out[b] = acc.T + b_out + vbias
    return out
